# revision 28
# baseline (speedup 1.0000x reference)
"""Trainium2 Bass kernel for BasicInteractionNetworkModule.

Data-parallel over batch (B=16) across 8 NeuronCores, 2 batches/core.

Math (per batch b):
  senders   = S^T @ O          [R, 128]   (S = sender_relations [128, R])
  receivers = R_rel^T @ O      [R, 128]
  rel_x = [senders, receivers, info]   [R, 320]
  h = relu-MLP(rel_x): 320 -> 256 -> 256 -> 256 -> 128 (relu after every layer)
  eff_recv = R_rel @ effects   [128, 128]
  obj_x = [O, ext, eff_recv]   [128, 288]
  out = relu-MLP2(obj_x): 288 -> 256 -> 256 -> 128 (no final relu)

Device strategy: keep relation-MLP activations feature-major (H^T, partition =
feature) so every layer is out^T = W^T @ H^T with stationary weights and the
bias+ReLU lands on ACT/DVE as a per-partition scalar. Layer-1 folds the
sender/receiver projection in via A_s = O @ rw1[:128], A_r = O @ rw1[128:256]
so S and R stream straight from DRAM as the moving operand. L4 + the
aggregation run in bf16 (free dim 128 would be 4x slower in fp32r); effects
are flipped relation-major with one xbar DMA-transpose per chunk.
"""

import numpy as np
import ml_dtypes

B, N_OBJ, N_REL = 16, 128, 16256
OBJ_D, REL_D, EFF_D, EXT_D, OUT_D = 128, 64, 128, 32, 128
HID = 256
N_CORES = 8
B_CORE = B // N_CORES  # 2
M_CHUNK = 1024

_CACHE = {}


def _chunks():
    out = []
    base = 0
    while base < N_REL:
        mc = min(M_CHUNK, N_REL - base)
        out.append((base, mc))
        base += mc
    return out


def _mtiles(mc):
    out = []
    base = 0
    while base < mc:
        n = min(512, mc - base)
        out.append((base, n))
        base += n
    return out


def build_kernel():
    from concourse import bacc
    import concourse.mybir as mybir
    import concourse.tile as tile
    from concourse.tile_rust import add_dep_helper

    F32 = mybir.dt.float32
    F32R = mybir.dt.float32r
    BF16 = mybir.dt.bfloat16
    RELU = mybir.ActivationFunctionType.Relu
    ADD = mybir.AluOpType.add
    MAX = mybir.AluOpType.max

    nc = bacc.Bacc(None)

    # per-core inputs
    S_d = nc.dram_tensor("s_rel", [B_CORE, N_OBJ, N_REL], BF16, kind="ExternalInput")
    R_d = nc.dram_tensor("r_rel", [B_CORE, N_OBJ, N_REL], BF16, kind="ExternalInput")
    IT_d = nc.dram_tensor("info_t", [B_CORE, REL_D, N_REL], BF16, kind="ExternalInput")
    RT_d = nc.dram_tensor("r_rel_t", [B_CORE, N_REL, N_OBJ], BF16, kind="ExternalInput")
    OT_d = nc.dram_tensor("objs_t", [B_CORE, OBJ_D, N_OBJ], F32R, kind="ExternalInput")
    XT_d = nc.dram_tensor("ext_t", [B_CORE, EXT_D, N_OBJ], F32R, kind="ExternalInput")

    rw1s_d = nc.dram_tensor("rw1s", [128, HID], F32R, kind="ExternalInput")
    rw1r_d = nc.dram_tensor("rw1r", [128, HID], F32R, kind="ExternalInput")
    rw1i_d = nc.dram_tensor("rw1i", [64, HID], BF16, kind="ExternalInput")
    rw2_d = nc.dram_tensor("rw2f", [128, 2, HID], BF16, kind="ExternalInput")
    rw3_d = nc.dram_tensor("rw3f", [128, 2, HID], BF16, kind="ExternalInput")
    rw4_d = nc.dram_tensor("rw4b", [128, 2, EFF_D], BF16, kind="ExternalInput")
    ow1o_d = nc.dram_tensor("ow1o", [128, HID], F32R, kind="ExternalInput")
    ow1x_d = nc.dram_tensor("ow1x", [EXT_D, HID], F32R, kind="ExternalInput")
    ow1e_d = nc.dram_tensor("ow1e", [128, HID], F32R, kind="ExternalInput")
    ow2_d = nc.dram_tensor("ow2f", [128, 2, HID], F32R, kind="ExternalInput")
    ow3_d = nc.dram_tensor("ow3f", [128, 2, OUT_D], F32R, kind="ExternalInput")

    rb1_d = nc.dram_tensor("rb1c", [128, 2], F32, kind="ExternalInput")
    rb2_d = nc.dram_tensor("rb2c", [128, 2], F32, kind="ExternalInput")
    rb3_d = nc.dram_tensor("rb3c", [128, 2], F32, kind="ExternalInput")
    rb4_d = nc.dram_tensor("rb4c", [128, 1], F32, kind="ExternalInput")
    ob1_d = nc.dram_tensor("ob1c", [128, 2], F32, kind="ExternalInput")
    ob2_d = nc.dram_tensor("ob2c", [128, 2], F32, kind="ExternalInput")
    ob3_d = nc.dram_tensor("ob3r", [128, OUT_D], F32, kind="ExternalInput")
    rb4r_d = nc.dram_tensor("rb4r", [1, 512], BF16, kind="ExternalInput")

    out_d = nc.dram_tensor("out", [B_CORE, N_OBJ, OUT_D], F32, kind="ExternalOutput")

    with tile.TileContext(nc) as tc:
        with (
            tc.tile_pool(name="wts", bufs=1) as wts,
            tc.tile_pool(name="perb", bufs=2) as perb,
            tc.tile_pool(name="cin", bufs=4) as cin,
            tc.tile_pool(name="acts", bufs=3) as acts,
            tc.tile_pool(name="psB", bufs=3, space="PSUM") as psB,
            tc.tile_pool(name="psS", bufs=2, space="PSUM") as psS,
        ):
            # ---- persistent weights ----
            rw1s = wts.tile([128, HID], F32R)
            rw1r = wts.tile([128, HID], F32R)
            rw1i = wts.tile([64, HID], BF16)
            rw2 = wts.tile([128, 2, HID], BF16)
            rw3 = wts.tile([128, 2, HID], BF16)
            rw4 = wts.tile([128, 2, EFF_D], BF16)
            ow1o = wts.tile([128, HID], F32R)
            ow1x = wts.tile([EXT_D, HID], F32R)
            ow1e = wts.tile([128, HID], F32R)
            ow2 = wts.tile([128, 2, HID], F32R)
            ow3 = wts.tile([128, 2, OUT_D], F32R)
            rb1 = wts.tile([128, 2], F32)
            rb2 = wts.tile([128, 2], F32)
            rb3 = wts.tile([128, 2], F32)
            rb4 = wts.tile([128, 1], F32)
            ob1 = wts.tile([128, 2], F32)
            ob2 = wts.tile([128, 2], F32)
            ob3 = wts.tile([128, OUT_D], F32)
            rb4r = wts.tile([1, 512], BF16)
            ones1 = wts.tile([1, 128], BF16)
            nc.vector.memset(ones1, 1.0)
            nc.sync.dma_start(rw1s, rw1s_d[:])
            nc.sync.dma_start(rw1r, rw1r_d[:])
            nc.sync.dma_start(rw1i, rw1i_d[:])
            nc.sync.dma_start(rb1, rb1_d[:])
            for t, d in [(rw2, rw2_d), (rw3, rw3_d), (rw4, rw4_d),
                         (ow1o, ow1o_d), (ow1x, ow1x_d), (ow1e, ow1e_d),
                         (ow2, ow2_d), (ow3, ow3_d),
                         (rb2, rb2_d), (rb3, rb3_d), (rb4, rb4_d),
                         (ob1, ob1_d), (ob2, ob2_d), (ob3, ob3_d),
                         (rb4r, rb4r_d)]:
                nc.gpsimd.dma_start(t, d[:])

            # ---- per-batch setup, hoisted for both batches so batch 1's
            # relation pipeline is not gated behind batch 0's object MLP ----
            OTs, XTs, Ass, Ars = [], [], [], []
            for b in range(B_CORE):
                OT = perb.tile([OBJ_D, N_OBJ], F32R, tag=f"OT{b}")
                XT = perb.tile([EXT_D, N_OBJ], F32R, tag=f"XT{b}")
                nc.sync.dma_start(OT, OT_d[b])
                nc.gpsimd.dma_start(XT, XT_d[b])
                As = perb.tile([N_OBJ, HID], BF16, tag=f"As{b}")
                Ar = perb.tile([N_OBJ, HID], BF16, tag=f"Ar{b}")
                psA = psS.tile([128, HID], F32, tag="sm")
                nc.tensor.matmul(psA, OT, rw1s, start=True, stop=True)
                nc.vector.tensor_copy(As, psA)
                psA2 = psS.tile([128, HID], F32, tag="sm")
                nc.tensor.matmul(psA2, OT, rw1r, start=True, stop=True)
                nc.vector.tensor_copy(Ar, psA2)
                OTs.append(OT); XTs.append(XT); Ass.append(As); Ars.append(Ar)

            for b in range(B_CORE):
                OT, XT, As, Ar = OTs[b], XTs[b], Ass[b], Ars[b]
                effT = perb.tile([EFF_D, N_OBJ], F32R, tag="effT")

                def stageA(ci, base, mc, st):
                    ns = mc // 128
                    S_c = cin.tile([N_OBJ, M_CHUNK], BF16, tag="S_c")
                    R_c = cin.tile([N_OBJ, M_CHUNK], BF16, tag="R_c")
                    I_c = cin.tile([REL_D, M_CHUNK], BF16, tag="I_c")
                    RT_c = cin.tile([128, M_CHUNK // 128, N_OBJ], BF16, tag="RT_c")
                    nc.sync.dma_start(S_c[:, :mc], S_d[b, :, base:base + mc])
                    nc.sync.dma_start(R_c[:, :mc], R_d[b, :, base:base + mc])
                    nc.sync.dma_start(I_c[:, :mc], IT_d[b, :, base:base + mc])
                    nc.sync.dma_start(
                        RT_c[:, :ns, :],
                        RT_d[b, base:base + mc, :].rearrange("(s p) o -> p s o", p=128),
                    )
                    H1 = acts.tile([128, 2, M_CHUNK], BF16, tag="H1")
                    H2 = acts.tile([128, 2, M_CHUNK], BF16, tag="H2")
                    for p2 in range(2):
                        ps = psB.tile([128, M_CHUNK], F32, tag="ps")
                        h = slice(p2 * 128, (p2 + 1) * 128)
                        for mt, n in _mtiles(mc):
                            sl = slice(mt, mt + n)
                            nc.tensor.matmul(ps[:, sl], rw1i[:, h], I_c[:, sl], start=True, stop=False)
                            nc.tensor.matmul(ps[:, sl], As[:, h], S_c[:, sl], start=False, stop=False)
                            nc.tensor.matmul(ps[:, sl], Ar[:, h], R_c[:, sl], start=False, stop=True)
                        hmc = min(512, mc)
                        nc.scalar.activation(H1[:, p2, :hmc], ps[:, :hmc], RELU,
                                             bias=rb1[:, p2:p2 + 1], scale=1.0)
                        if mc > 512:
                            nc.vector.tensor_scalar(H1[:, p2, 512:mc], ps[:, 512:mc],
                                                    rb1[:, p2:p2 + 1], 0.0, ADD, MAX)
                    for p2 in range(2):
                        ps = psB.tile([128, M_CHUNK], F32, tag="ps")
                        h = slice(p2 * 128, (p2 + 1) * 128)
                        for mt, n in _mtiles(mc):
                            sl = slice(mt, mt + n)
                            nc.tensor.matmul(ps[:, sl], rw2[:, 0, h], H1[:, 0, sl], start=True, stop=False)
                            nc.tensor.matmul(ps[:, sl], rw2[:, 1, h], H1[:, 1, sl], start=False, stop=True)
                        hmc = min(512, mc)
                        nc.scalar.activation(H2[:, p2, :hmc], ps[:, :hmc], RELU,
                                             bias=rb2[:, p2:p2 + 1], scale=1.0)
                        if mc > 512:
                            nc.vector.tensor_scalar(H2[:, p2, 512:mc], ps[:, 512:mc],
                                                    rb2[:, p2:p2 + 1], 0.0, ADD, MAX)
                    st['H2'] = H2
                    st['RT_c'] = RT_c

                def stageB(ci, base, mc, st):
                    ns = mc // 128
                    H2 = st['H2']; RT_c = st['RT_c']
                    H3 = acts.tile([128, 2, M_CHUNK], BF16, tag="H3")
                    E3 = acts.tile([128, M_CHUNK], BF16, tag="E3")
                    for p2 in range(2):
                        ps = psB.tile([128, M_CHUNK], F32, tag="ps")
                        h = slice(p2 * 128, (p2 + 1) * 128)
                        for mt, n in _mtiles(mc):
                            sl = slice(mt, mt + n)
                            nc.tensor.matmul(ps[:, sl], rw3[:, 0, h], H2[:, 0, sl], start=True, stop=False)
                            nc.tensor.matmul(ps[:, sl], rw3[:, 1, h], H2[:, 1, sl], start=False, stop=True)
                        hmc = min(512, mc)
                        nc.vector.tensor_scalar(H3[:, p2, :hmc], ps[:, :hmc],
                                                rb3[:, p2:p2 + 1], 0.0, ADD, MAX)
                        if mc > 512:
                            nc.scalar.activation(H3[:, p2, 512:mc], ps[:, 512:mc], RELU,
                                                 bias=rb3[:, p2:p2 + 1], scale=1.0)
                    for g in range(0, ns, 4):
                        ge = min(g + 4, ns)
                        span = (ge - g) * 128
                        gsl = slice(g * 128, g * 128 + span)
                        ps4 = psS.tile([128, 512], F32, tag="sm")
                        bias_mm = nc.tensor.matmul(
                            ps4[:, :span], ones1, rb4r[:, :span],
                            start=True, stop=False, skip_group_check=True)
                        for sj in range(g, ge):
                            sl = slice(sj * 128, (sj + 1) * 128)
                            psl = slice((sj - g) * 128, (sj - g + 1) * 128)
                            m0 = nc.tensor.matmul(ps4[:, psl], H3[:, 0, sl], rw4[:, 0, :],
                                                  start=False, stop=False,
                                                  skip_group_check=True)
                            add_dep_helper(m0.ins, bias_mm.ins, sync=False,
                                           reason="bias seeds psum before k-mms")
                            nc.tensor.matmul(ps4[:, psl], H3[:, 1, sl], rw4[:, 1, :],
                                             start=False, stop=(sj == ge - 1),
                                             skip_group_check=True)
                        nc.vector.tensor_scalar_max(E3[:, gsl], ps4[:, :span], 0.0)
                    psa = psS.tile([128, N_OBJ], F32, tag="sm")
                    for sj in range(ns):
                        nc.tensor.matmul(psa, E3[:, sj * 128:(sj + 1) * 128],
                                         RT_c[:, sj, :],
                                         start=(sj == 0), stop=(sj == ns - 1))
                    if ci == 0:
                        nc.vector.tensor_copy(effT, psa)
                    else:
                        nc.vector.tensor_tensor(effT, effT, psa, mybir.AluOpType.add)

                # 2-stage software pipeline: emit chunk ci's front half, then
                # chunk ci-1's back half, so PE has fill work during evac waits
                chs = _chunks()
                sts = [dict() for _ in chs]
                for ci, (base, mc) in enumerate(chs):
                    stageA(ci, base, mc, sts[ci])
                    if ci > 0:
                        stageB(ci - 1, chs[ci - 1][0], chs[ci - 1][1], sts[ci - 1])
                ci = len(chs) - 1
                stageB(ci, chs[ci][0], chs[ci][1], sts[ci])

                # ---- object MLP ----
                G1 = perb.tile([128, 2, N_OBJ], F32R, tag="G1")
                G2 = perb.tile([128, 2, N_OBJ], F32R, tag="G2")
                for p2 in range(2):
                    ps = psS.tile([128, N_OBJ], F32, tag="sm")
                    h = slice(p2 * 128, (p2 + 1) * 128)
                    nc.tensor.matmul(ps, ow1o[:, h], OT, start=True, stop=False)
                    nc.tensor.matmul(ps, ow1x[:, h], XT, start=False, stop=False)
                    nc.tensor.matmul(ps, ow1e[:, h], effT, start=False, stop=True)
                    nc.scalar.activation(G1[:, p2, :], ps, RELU,
                                         bias=ob1[:, p2:p2 + 1], scale=1.0)
                for p2 in range(2):
                    ps = psS.tile([128, N_OBJ], F32, tag="sm")
                    h = slice(p2 * 128, (p2 + 1) * 128)
                    nc.tensor.matmul(ps, ow2[:, 0, h], G1[:, 0, :], start=True, stop=False)
                    nc.tensor.matmul(ps, ow2[:, 1, h], G1[:, 1, :], start=False, stop=True)
                    nc.scalar.activation(G2[:, p2, :], ps, RELU,
                                         bias=ob2[:, p2:p2 + 1], scale=1.0)
                # final layer, output object-major: out[o, :] = G2^T chunks
                pso = psS.tile([N_OBJ, OUT_D], F32, tag="sm")
                nc.tensor.matmul(pso, G2[:, 0, :], ow3[:, 0, :], start=True, stop=False)
                nc.tensor.matmul(pso, G2[:, 1, :], ow3[:, 1, :], start=False, stop=True)
                ob = perb.tile([N_OBJ, OUT_D], F32, tag="ob")
                nc.vector.tensor_tensor(ob, pso, ob3, mybir.AluOpType.add)
                nc.sync.dma_start(out_d[b], ob)

    nc.compile()
    return nc


def _prep_inputs(objects, sender_relations, receiver_relations, relation_info,
                 external_effect_info, rw1, rb1, rw2, rb2, rw3, rb3, rw4, rb4,
                 ow1, ob1, ow2, ob2, ow3, ob3):
    bf16 = ml_dtypes.bfloat16
    f32 = np.float32

    def a(x):
        return np.ascontiguousarray(np.asarray(x, dtype=f32))

    objects = a(objects); sender_relations = a(sender_relations)
    receiver_relations = a(receiver_relations); relation_info = a(relation_info)
    external_effect_info = a(external_effect_info)
    rw1, rb1, rw2, rb2, rw3, rb3, rw4, rb4 = map(a, (rw1, rb1, rw2, rb2, rw3, rb3, rw4, rb4))
    ow1, ob1, ow2, ob2, ow3, ob3 = map(a, (ow1, ob1, ow2, ob2, ow3, ob3))

    info_t_bf = np.ascontiguousarray(relation_info.transpose(0, 2, 1)).astype(bf16)
    s_bf = sender_relations.astype(bf16)
    r_bf = receiver_relations.astype(bf16)
    r_rel_t = np.ascontiguousarray(
        receiver_relations.transpose(0, 2, 1)).astype(bf16)
    objs_t = np.ascontiguousarray(objects.transpose(0, 2, 1))
    ext_t = np.ascontiguousarray(external_effect_info.transpose(0, 2, 1))

    shared = {
        "rw1s": rw1[0:128].copy(),
        "rw1r": rw1[128:256].copy(),
        "rw1i": rw1[256:320].astype(bf16),
        "rw2f": np.ascontiguousarray(rw2.reshape(2, 128, HID).transpose(1, 0, 2)).astype(bf16),
        "rw3f": np.ascontiguousarray(rw3.reshape(2, 128, HID).transpose(1, 0, 2)).astype(bf16),
        "rw4b": np.ascontiguousarray(rw4.reshape(2, 128, EFF_D).transpose(1, 0, 2)).astype(bf16),
        "ow1o": ow1[0:128].copy(),
        "ow1x": ow1[128:160].copy(),
        "ow1e": ow1[160:288].copy(),
        "ow2f": np.ascontiguousarray(ow2.reshape(2, 128, HID).transpose(1, 0, 2)),
        "ow3f": np.ascontiguousarray(ow3.reshape(2, 128, OUT_D).transpose(1, 0, 2)),
        "rb1c": np.ascontiguousarray(rb1.reshape(2, 128).T),
        "rb2c": np.ascontiguousarray(rb2.reshape(2, 128).T),
        "rb3c": np.ascontiguousarray(rb3.reshape(2, 128).T),
        "rb4c": np.ascontiguousarray(rb4.reshape(128, 1)),
        "ob1c": np.ascontiguousarray(ob1.reshape(2, 128).T),
        "ob2c": np.ascontiguousarray(ob2.reshape(2, 128).T),
        "ob3r": np.ascontiguousarray(np.broadcast_to(ob3[None, :], (128, OUT_D))),
        "rb4r": np.ascontiguousarray(np.tile(rb4, 4)[None, :]).astype(ml_dtypes.bfloat16),
    }

    in_maps = []
    for c in range(N_CORES):
        sl = slice(c * B_CORE, (c + 1) * B_CORE)
        m = dict(shared)
        m["s_rel"] = s_bf[sl]
        m["r_rel"] = r_bf[sl]
        m["info_t"] = info_t_bf[sl]
        m["r_rel_t"] = r_rel_t[sl]
        m["objs_t"] = objs_t[sl]
        m["ext_t"] = ext_t[sl]
        in_maps.append(m)
    return in_maps


def run(in_maps, **spmd_kwargs):
    from concourse.bass_utils import run_bass_kernel_spmd

    if "nc" not in _CACHE:
        _CACHE["nc"] = build_kernel()
    return run_bass_kernel_spmd(_CACHE["nc"], in_maps,
                                core_ids=list(range(N_CORES)), **spmd_kwargs)


def kernel(**inputs) -> np.ndarray:
    in_maps = _prep_inputs(**inputs)
    res = run(in_maps)
    out = np.concatenate([r["out"].reshape(-1, OUT_D) for r in res.results], axis=0)
    return np.ascontiguousarray(out, dtype=np.float32)
